# revision 39
# baseline (speedup 1.0000x reference)
"""Trainium2 Bass kernel for nn_KeyedConv2d: 3x3 SAME conv, stride 1.

x: [8, 64, 64, 64] (NCHW), Wt: [64, 64, 3, 3] (OIHW) -> out [8, 64, 64, 64].

Sharding: data-parallel over batch, one image per NeuronCore (8 cores).

Per-core algorithm (v4):
- Matmul cost on TRN2 is output_free_size x cycle, independent of
  contraction depth and output partition count.  OC=64 uses only half the
  PE's 128 output columns, so each matmul computes TWO adjacent output
  pixels per column: out partitions 0-63 = pixel 2c ("u=0"), 64-127 =
  pixel 2c+1 ("u=1").  SBUF partitions 0-63 hold the padded image,
  partitions 64-127 hold it shifted one column; a stride-2 column read at
  base b0 then provides taps pad[2c+b0] (s=0) and pad[2c+b0+1] (s=1).
  Tap algebra kx = b0 + s - u gives full 3x3 coverage with just TWO
  256-column matmuls per ky (b0 in {0,2}; invalid slots get zero weights):
  6 matmuls x 256 cols per 512-px chunk = 1536 PE cycles, vs 4608 for the
  naive 9x512 scheme.  The pixel-interleaved output is de-interleaved on
  the host for free.
- ~19 warmup matmuls on a zeroed tile keep the PE pstate ramp going from
  ~1us so every real matmul is costed at the full 2.4 GHz clock;
  instruction cost is fixed at dep-ready time.
- The image is pre-cast to bf16 on the host and streamed as 8 per-strip
  HBM->SBUF DMAs (contiguous elements, partition_broadcast duplicates it
  into both partition halves; engines cannot cross partitions); the
  weights ride the gpsimd software DGE to stay off the serial HWDGE
  chain.
- Every pad copy sources from its piece (uniform dependencies keep the
  static scheduler from head-blocking engine queues).  Strips 0-1 (which
  gate the pipeline start) avoid the Activation engine entirely -- the
  scheduler's internal cost model rates Act copies late and would hoist
  later chunks in front.  DVE (2x 16-bit mode) carries tops + half the
  bottoms, Pool the other bottom half, Act the PSUM evictions.
- bf16 matmuls, f32 PSUM accumulation (rel err ~4e-3 vs 2e-2 budget), one
  half-bank [128,256] PSUM tile per chunk, bf16 evictions, batched
  stores.  Chunk 7 runs as 384+128-px half-chunks so only a 128-px
  eviction + store trails the last matmul.
"""
import numpy as np
import ml_dtypes

import concourse.bass as bass
import concourse.mybir as mybir
import concourse.tile as tile
from concourse import bacc
from concourse.bass_utils import run_bass_kernel_spmd

F32 = mybir.dt.float32
BF16 = mybir.dt.bfloat16

IC = OC = 64
H = W = 64
K = 3
HWPIX = H * W        # 4096
CHUNK = 512          # output pixels per chunk (one half-bank PSUM tile)
NCH = HWPIX // CHUNK  # 8 chunks
RPC = CHUNK // W     # 8 image rows per chunk
PW = W + 1           # padded row width (left zero col; col 64 of the last
                     # window doubles as right pad via row contiguity)
SROWS = RPC + 2      # padded rows per strip incl halo (10)
TLEN = SROWS * PW + 2  # 652

WARMN = 19           # warmup matmuls (PE pstate ramp cover until chunk 0)

MODE = "bf16"


def _build(mode: str = MODE) -> bacc.Bacc:
    nc = bacc.Bacc("TRN2", target_bir_lowering=False, debug=False)

    # image pre-cast to bf16 on the host: halves the piece-DMA bytes (the
    # matmuls consume bf16 anyway)
    xbf = nc.dram_tensor("xbf", [IC, H, W], BF16, kind="ExternalInput").ap()
    # host-prepacked paired-pixel weights [128, 6*128], see _pack_weights
    wt = nc.dram_tensor("wt", [128, 6 * 2 * OC], BF16, kind="ExternalInput").ap()
    # pixel-interleaved output: y[64u+oc, k*256 + r*32 + c] =
    #   out[oc, 8k+r, 2c+u]; host de-interleaves
    y = nc.dram_tensor("y", [128, HWPIX // 2], BF16, kind="ExternalOutput").ap()

    with tile.TileContext(nc) as tc:
        with (
            tc.tile_pool(name="wsb", bufs=1) as wsb_pool,
            tc.tile_pool(name="warm", bufs=1) as warm_pool,
            tc.tile_pool(name="piece", bufs=1) as piece_pool,
            tc.tile_pool(name="xs", bufs=1) as xs_pool,
            tc.tile_pool(name="psum", bufs=1, space="PSUM") as psum_pool,
            tc.tile_pool(name="osb", bufs=6) as osb_pool,
        ):
            # --- warmup source (DVE memset, ready ~0.8us)
            warm = warm_pool.tile([64, 128], BF16)
            nc.vector.memset(warm[:, :], 0.0)

            # --- warmup matmuls: keep the PE pstate ramp going until the
            # first real chunk is ready.  Shares the chunk-0 PSUM bank tag;
            # all deps are PE-program-order so the sharing is free.
            wps = psum_pool.tile([128, CHUNK // 2], F32, name="ps0")
            for _ in range(WARMN):
                nc.tensor.matmul(
                    wps[0:64, 0:128], warm[:, 0:64], warm[:, 0:128],
                    start=True, stop=True, skip_group_check=True,
                )

            # --- weights via SWDGE (gpsimd): stays off the serial HWDGE
            # chain so the piece DMAs pipeline back-to-back.  The leading
            # dummy memset delays the SWDGE descriptor-gen just enough that
            # the weight transfer's bus request lands AFTER piece 1's:
            # chunk 1 drives the PE finish time, chunk 0 has slack to absorb
            # the weight wait instead.
            wdly = warm_pool.tile([64, 420], BF16, name="wdly")
            nc.gpsimd.memset(wdly[:, :], 0.0)
            wsb = wsb_pool.tile([128, 6 * 2 * OC], BF16)
            nc.gpsimd.dma_start(wsb[:, :], wt)

            # --- staging pieces: strip k needs image rows 8k-1 .. 8k+8
            pieces = {}
            prows = {}
            for k in range(0, NCH):
                r0 = max(8 * k - 1, 0)
                r1 = min(8 * k + 8, H - 1)
                nr = r1 - r0 + 1
                pc = piece_pool.tile([128, nr * W], BF16, name=f"pc{k}")
                nc.sync.dma_start(
                    pc[:, :], xbf[:, r0:r1 + 1, :].partition_broadcast(2)
                )
                pieces[k] = pc
                prows[k] = (r0, nr)

            # --- strip tiles + zero slivers (DVE, all early: no deps)
            xss = []
            for k in range(NCH):
                xs = xs_pool.tile([128, TLEN], BF16, name=f"xs{k}")
                xss.append(xs)
                # top half left-pad column (rows 0..9 plus the trailing
                # element 650 = "row 10 col 0")
                nc.vector.memset(
                    xs[0:64, 0:SROWS * PW].rearrange(
                        "p (a b) -> p a b", b=PW)[:, :, 0:1],
                    0.0,
                )
                nc.vector.memset(xs[0:64, SROWS * PW:SROWS * PW + 1], 0.0)
                # bottom col 64 = left pad of the next row as seen from the
                # shifted half; read by the b0=2 matmuls at c=31
                nc.vector.memset(
                    xs[64:128, 0:SROWS * PW].rearrange(
                        "p (a b) -> p a b", b=PW)[:, :, W:PW],
                    0.0,
                )
                if k == 0:
                    nc.vector.memset(xs[:, 0:PW], 0.0)          # pad row 0
                if k == NCH - 1:
                    nc.vector.memset(xs[:, 9 * PW:9 * PW + PW], 0.0)  # row 65

            # Pad-copy jobs (strip, piece, rows, half, engine).  Top: cols
            # 1..64 <- img cols 0..63; bottom: cols 0..63 <- img (one col
            # left-shifted).  Every job depends only on its piece DMA.
            # Strips 0-1 gate the pipeline start (finely split, DVE-heavy);
            # steady state: DVE does the top + bottom rows 0-4 (2x 16-bit
            # mode), Pool bottom rows 5-9, Act only evictions.
            T, B = 0, 1
            copy_jobs = [
                (0, 0, 1, 9, T, nc.vector), (0, 0, 1, 9, B, nc.vector),
                (1, 1, 0, 9, T, nc.vector), (1, 1, 0, 9, B, nc.gpsimd),
            ]
            for k in range(2, NCH):
                rh = 9 if k < NCH - 1 else 8
                copy_jobs += [
                    (k, k, 0, rh, T, nc.vector),
                    (k, k, 0, 4, B, nc.vector),
                    (k, k, 5, rh, B, nc.gpsimd),
                ]

            for k, key, rlo, rhi, half, eng in copy_jobs:
                pc = pieces[key]
                xs = xss[k]
                r0, nr = prows[key]
                # strip row r holds padded row 8k+r = img row 8k+r-1
                off = 8 * k + rlo - 1 - r0
                assert 0 <= off and off + (rhi - rlo) < nr
                src = pc[:, off * W:(off + rhi - rlo + 1) * W].rearrange(
                    "p (a b) -> p a b", b=W)
                dst = xs[:, rlo * PW:(rhi + 1) * PW].rearrange(
                    "p (a b) -> p a b", b=PW)
                if half == T:
                    if eng is nc.scalar:
                        eng.copy(dst[0:64, :, 1:1 + W], src[0:64, :, :])
                    else:
                        eng.tensor_copy(dst[0:64, :, 1:1 + W], src[0:64, :, :])
                else:
                    eng.tensor_copy(dst[64:128, :, 0:W], src[64:128, :, :])

            # --- conv: per chunk, 6 paired-pixel matmuls (b0 in {0,2} per
            # ky) accumulate into one [128, npix/2] PSUM tile.  Chunk 7 is
            # split into 384+128-px halves (recycling the ps0/ps1 tags,
            # long since evicted) so the post-last-matmul tail is short.
            # store groups: (c0,c1), (c2,c3), (c4,c5,c6), (7a,7b) -- the
            # final store is small and its HWDGE slot never queues behind
            # another store after the last matmul
            work = [
                (k, 0, CHUNK, f"ps{k}", k in (1, 3, 6))
                for k in range(NCH - 1)
            ]
            work += [(7, 0, 6 * W, "ps0", False), (7, 6, 2 * W, "ps1", True)]
            evicted = []
            for k, rbase, npix, tag, flush in work:
                xs = xss[k]
                nrows = npix // W
                ps = psum_pool.tile([128, npix // 2], F32, name=tag)
                t = 0
                for ky in range(K):
                    for b0 in (0, 2):
                        bb = (rbase + ky) * PW + b0
                        rhs = xs[:, bb:bb + nrows * PW].rearrange(
                            "p (a b) -> p a b", b=PW)[:, :, 0:W].rearrange(
                            "p a (c t) -> p a c t", t=2)[:, :, :, 0:1]
                        m = ky * 2 + (b0 // 2)
                        nc.tensor.matmul(
                            ps[:, :], wsb[:, m * 128:(m + 1) * 128], rhs,
                            start=(t == 0), stop=(t == 5),
                            skip_group_check=True,
                        )
                        t += 1
                # PSUM -> SBUF bf16 eagerly per chunk (Act; DVE for the
                # final 128-px half so it parallels Act's 384-px one);
                # batched HBM store at flush points
                if not evicted:
                    gbase = k * (CHUNK // 2) + rbase * (W // 2)
                    osb = osb_pool.tile([128, 3 * CHUNK // 2], BF16, name="osb")
                odst = osb[:, sum(evicted):sum(evicted) + npix // 2]
                if (tag == "ps1" and k == 7) or k == 6:
                    nc.vector.tensor_copy(odst, ps[:, :])
                else:
                    nc.scalar.copy(odst, ps[:, :])
                evicted.append(npix // 2)
                if flush:
                    tot = sum(evicted)
                    nc.sync.dma_start(
                        y[:, gbase:gbase + tot], osb[:, 0:tot]
                    )
                    evicted = []

    nc.compile()
    return nc


_NC_CACHE: dict[str, bacc.Bacc] = {}


def _pack_weights(Wt: np.ndarray) -> np.ndarray:
    """Paired-pixel weight packing.

    Block m = ky*2 + (b0//2), lhsT[(ic, s), 64u + oc]: the (u, s) slot of
    block (ky, b0) carries W[oc, ic, ky, kx] for kx = b0 + s - u, or zero
    when kx is out of range.
    """
    Wf = Wt.astype(np.float32)
    wsb = np.zeros((128, 6 * 2 * OC), dtype=np.float32)
    for ky in range(K):
        for b0 in (0, 2):
            m = ky * 2 + (b0 // 2)
            for u in (0, 1):
                for s in (0, 1):
                    kx = b0 + s - u
                    if 0 <= kx < K:
                        wsb[64 * s:64 * s + 64,
                            m * 128 + 64 * u:m * 128 + 64 * u + 64] = (
                            Wf[:, :, ky, kx].T
                        )
    return wsb.astype(ml_dtypes.bfloat16)


def kernel(x: np.ndarray, Wt: np.ndarray) -> np.ndarray:
    assert x.shape == (8, IC, H, W) and Wt.shape == (OC, IC, K, K)
    if MODE not in _NC_CACHE:
        _NC_CACHE[MODE] = _build(MODE)
    nc = _NC_CACHE[MODE]

    wt_t = _pack_weights(Wt)
    in_maps = [
        {
            "xbf": np.ascontiguousarray(x[b].astype(ml_dtypes.bfloat16)),
            "wt": wt_t,
        }
        for b in range(8)
    ]
    global _last_in_maps
    _last_in_maps = in_maps
    res = run_bass_kernel_spmd(nc, in_maps, core_ids=list(range(8)))
    # de-interleave: y[64u+oc, k*256 + r*32 + c] = out[oc, 8k+r, 2c+u]
    outs = []
    for r in res.results:
        yv = np.asarray(r["y"]).reshape(2, OC, NCH, RPC, W // 2)
        outs.append(yv.transpose(1, 2, 3, 4, 0).reshape(OC, H, W))
    return np.stack(outs).astype(np.float32)


_last_in_maps: list[dict[str, np.ndarray]] = []
